# revision 6
# baseline (speedup 1.0000x reference)
"""Trainium2 Bass kernel for nn_AttentionModel (sparse_attention).

8-core tensor-parallel distribution with a software-pipelined schedule:
 - k/v layer-1 convs run FIRST; their whole downstream chain (ReduceScatter,
   AllGather, layer-2/3 convs, pw@v3 fold) hides under the q1 conv pass.
 - q1 is split into three row-blocks (16/8/8 of 32 H-rows). After each block,
   its q2 partials + chunked ReduceScatter + epilogue + chunked AllGather +
   q3 rows + chunked AllGather + partial scores pipeline under the next block.
 - the 1x1 output projection is folded into v3 (pv3 = pw @ v3, computed
   mid-kernel), so the tail after softmax is just 8 matmuls + bias + DMA.

dtype strategy: fp16 for all conv inputs/weights/activations and collective
payloads (half the HBM + wire bytes of fp32, same 1 cyc/row PE rate), fp32
PSUM accumulation and fp32 softmax. ~1.8e-3 max rel err vs fp32 reference.
"""
import os
import sys
import numpy as np

for _p in ('/opt/trn_rl_repo', '/root/problem/work'):
    if _p not in sys.path:
        sys.path.insert(0, _p)

import concourse.bass as bass
import concourse.bacc as bacc
import concourse.tile as tile
import concourse.mybir as mybir
from concourse import bass_utils
from concourse.bass_interp import get_hw_module

F32 = mybir.dt.float32
F16 = mybir.dt.float16
I32 = mybir.dt.int32
AF = mybir.ActivationFunctionType
ALU = mybir.AluOpType
AX = mybir.AxisListType

NCORES = 8
_CACHE = {}

# q1 out-row blocks; derived q2/q3 row blocks and shipped padded-row ranges
Q1B = [(0, 16), (16, 24), (24, 32)]
Q2B = [(0, 15), (15, 23), (23, 32)]     # q2 out rows computable after q1 block k
PRB = [(0, 16), (16, 24), (24, 34)]     # q2o padded rows shipped by ag2b block k
Q3B = [(0, 14), (14, 22), (22, 32)]     # q3 out rows computable after ag2b block k


def _rc(s0, s1):
    r = s0
    while r < s1:
        yield (r, min(r + 8, s1))
        r += 8


def _lrelu(nc, sb, src_ap, bias_ap, bias3_ap, out_ap, name):
    """out = max(src + b, 0.3*src + 0.3b)  (LeakyReLU 0.3)."""
    P = src_ap.shape[0]
    free = int(np.prod(src_ap.shape[1:]))
    s = sb.tile([P, free], F32, name=f"{name}_s", tag="epi_s")
    t = sb.tile([P, free], F32, name=f"{name}_t", tag="epi_t")
    nc.scalar.activation(s[:], src_ap, AF.Identity, bias=bias_ap, scale=1.0)
    nc.scalar.activation(t[:], src_ap, AF.Identity, bias=bias3_ap, scale=0.3)
    nc.vector.tensor_tensor(out_ap, s[:], t[:], op=ALU.max)


def build_program():
    nc = bacc.Bacc("TRN2", target_bir_lowering=False, debug=False,
                   enable_asserts=True, num_devices=NCORES)

    xpad_d = nc.dram_tensor("xpad", [16, 128, 34 * 66], F16, kind="ExternalInput")
    xdec_d = nc.dram_tensor("xdec", [16, 128, 4 * 17 * 33], F16, kind="ExternalInput")
    w1q_d = nc.dram_tensor("w1q", [16, 128, 1152], F16, kind="ExternalInput")
    w1k_d = nc.dram_tensor("w1k", [16, 128, 1152], F16, kind="ExternalInput")
    w1v_d = nc.dram_tensor("w1v", [16, 128, 1152], F16, kind="ExternalInput")
    w2q_d = nc.dram_tensor("w2q", [128, 2304], F16, kind="ExternalInput")
    w2k_d = nc.dram_tensor("w2k", [128, 2304], F16, kind="ExternalInput")
    w2v_d = nc.dram_tensor("w2v", [2, 128, 4608], F16, kind="ExternalInput")
    w3q_d = nc.dram_tensor("w3q", [2, 128, 288], F16, kind="ExternalInput")
    w3k_d = nc.dram_tensor("w3k", [2, 128, 288], F16, kind="ExternalInput")
    w3v_d = nc.dram_tensor("w3v", [8, 128, 1152], F16, kind="ExternalInput")
    wp_d = nc.dram_tensor("wp", [8, 128, 1024], F16, kind="ExternalInput")
    bias_d = nc.dram_tensor("bias", [128, 28], F32, kind="ExternalInput")
    bidx_d = nc.dram_tensor("bidx", [65, 1], I32, kind="ExternalInput")
    out_d = nc.dram_tensor("out_shard", [1024, 256], F32, kind="ExternalOutput")
    ident_d = nc.inline_tensor(np.eye(128, dtype=np.float32), name="ident")

    RG = [list(range(NCORES))]

    with tile.TileContext(nc) as tc:
        with (
            tc.tile_pool(name="dram", bufs=1, space="DRAM") as dram,
            tc.tile_pool(name="wpool", bufs=2) as wpool,
            tc.tile_pool(name="xpool", bufs=2) as xpool,
            tc.tile_pool(name="opool", bufs=1) as opool,
            tc.tile_pool(name="ppool", bufs=1, space="PSUM") as ppool,
            tc.tile_pool(name="misc", bufs=1) as misc,
        ):
            # ---------------- collective DRAM buffers (fp16) ----------------
            rsa_in = dram.tile([8, 17920], F16)
            rsa_out = dram.tile([17920], F16)
            ag2a_in = dram.tile([17920], F16)
            ag2a_out = dram.tile([8, 17920], F16, addr_space="Shared")
            agkv_in = dram.tile([32, 325], F16)
            agkv_out = dram.tile([256, 325], F16, addr_space="Shared")
            rsb_in, rsb_out, ag2b_in, ag2b_out, ag3_in, ag3_out = [], [], [], [], [], []
            for k in range(3):
                npos2 = (Q2B[k][1] - Q2B[k][0]) * 64
                nship = (PRB[k][1] - PRB[k][0]) * 66
                npos3 = (Q3B[k][1] - Q3B[k][0]) * 64
                rsb_in.append(dram.tile([8, 32 * npos2], F16, name=f"rsb_in{k}"))
                rsb_out.append(dram.tile([32 * npos2], F16, name=f"rsb_out{k}"))
                ag2b_in.append(dram.tile([32, nship], F16, name=f"ag2b_in{k}"))
                ag2b_out.append(dram.tile([256, nship], F16, name=f"ag2b_out{k}", addr_space="Shared"))
                ag3_in.append(dram.tile([32, npos3], F16, name=f"ag3_in{k}"))
                ag3_out.append(dram.tile([256, npos3], F16, name=f"ag3_out{k}", addr_space="Shared"))
            beta_dram = dram.tile([65, 2048], F16)

            biases = misc.tile([128, 28], F32)
            nc.sync.dma_start(biases[:], bias_d.ap())
            bcol = lambda j: biases[:, j:j + 1]
            ident = misc.tile([128, 128], F32)
            nc.sync.dma_start(ident[:], ident_d.ap())
            bidx = misc.tile([65, 1], I32)
            nc.sync.dma_start(bidx[:], bidx_d.ap())

            # warmup collective: pays first-collective setup during the kv pass
            warm_in = dram.tile([128, 4], F32)
            warm_out = dram.tile([1024, 4], F32, addr_space="Shared")
            nc.sync.dma_start(warm_in[:], bias_d.ap()[:, 0:4])
            nc.gpsimd.collective_compute("AllGather", ALU.bypass, replica_groups=RG,
                                         ins=[warm_in.opt()], outs=[warm_out.opt()])

            # resident weights (streamed on the scalar queue in need-order)
            w2k_sb = opool.tile([128, 2304], F16, name="w2k_sb")
            nc.scalar.dma_start(w2k_sb[:], w2k_d.ap())
            w2v_sb = opool.tile([128, 2 * 4608], F16, name="w2v_sb")
            for vh in range(2):
                nc.scalar.dma_start(w2v_sb[:, 4608 * vh:4608 * vh + 4608], w2v_d.ap()[vh])
            w1q_sb = opool.tile([128, 16 * 1152], F16, name="w1q_sb")
            for ic in range(4):
                nc.scalar.dma_start(w1q_sb[:, 1152 * ic:1152 * ic + 1152], w1q_d.ap()[ic])
            w3k_sb = opool.tile([128, 2 * 288], F16, name="w3k_sb")
            for jc in range(2):
                nc.scalar.dma_start(w3k_sb[:, 288 * jc:288 * jc + 288], w3k_d.ap()[jc])
            w3v_sb = opool.tile([128, 8 * 1152], F16, name="w3v_sb")
            for ic in range(8):
                nc.scalar.dma_start(w3v_sb[:, 1152 * ic:1152 * ic + 1152], w3v_d.ap()[ic])
            for ic in range(4, 16):
                nc.scalar.dma_start(w1q_sb[:, 1152 * ic:1152 * ic + 1152], w1q_d.ap()[ic])
            wp_sb = opool.tile([128, 8 * 1024], F16, name="wp_sb")
            for i in range(8):
                nc.scalar.dma_start(wp_sb[:, 1024 * i:1024 * i + 1024], wp_d.ap()[i])
            w2q_sb = opool.tile([128, 2304], F16, name="w2q_sb")
            nc.scalar.dma_start(w2q_sb[:], w2q_d.ap())
            w3q_sb = opool.tile([128, 2 * 288], F16, name="w3q_sb")
            for jc in range(2):
                nc.scalar.dma_start(w3q_sb[:, 288 * jc:288 * jc + 288], w3q_d.ap()[jc])

            # ---------------- persistent activation tiles --------------------
            q1_sb = opool.tile([128, 34 * 66], F16, name="q1_sb")
            q1o = q1_sb.rearrange("c (h w) -> c h w", h=34)
            q2_sb = opool.tile([32, 34 * 66], F16, name="q2_sb")
            q2o = q2_sb.rearrange("c (h w) -> c h w", h=34)
            q2full = opool.tile([128, 2 * 34 * 66], F16, name="q2full")
            q2f4 = q2full.rearrange("p (c h w) -> p c h w", c=2, h=34)
            scores_sb = opool.tile([65, 2048], F32, name="scores_sb")
            pv3sb = opool.tile([128, 520], F32, name="pv3sb")
            pv3T = opool.tile([65, 1024], F16, name="pv3T")
            k3f = opool.tile([128, 2 * 65], F16, name="k3f")
            v3fa = opool.tile([128, 8 * 65], F16, name="v3fa")
            pmax = [misc.tile([65, 1], F32, name=f"pmax{k}") for k in range(3)]

            # ================ phase KV: k1/v1 over all 16 ic chunks ==========
            k1_ps = ppool.tile([128, 512], F32, name="k1_ps", tag="pk")
            v1_ps = ppool.tile([128, 512], F32, name="v1_ps", tag="pv")
            for ic in range(16):
                xd = xpool.tile([128, 4 * 17 * 33], F16, name="xd", tag="xdec")
                nc.sync.dma_start(xd[:], xdec_d.ap()[ic])
                wk = wpool.tile([128, 1152], F16, name="wk", tag="wB")
                nc.sync.dma_start(wk[:], w1k_d.ap()[ic])
                wv = wpool.tile([128, 1152], F16, name="wv", tag="wC")
                nc.sync.dma_start(wv[:], w1v_d.ap()[ic])
                xd4 = xd.rearrange("c (f h w) -> c f h w", f=4, h=17)
                first, last = (ic == 0), (ic == 15)
                for tap in range(9):
                    dy, dx = tap // 3, tap % 3
                    ph = 2 * (dy % 2) + (dx % 2)
                    win2 = xd4[:, ph:ph + 1, dy // 2: dy // 2 + 16, dx // 2: dx // 2 + 32]
                    nc.tensor.matmul(k1_ps[:], wk[:, tap * 128:tap * 128 + 128], win2,
                                     start=(first and tap == 0), stop=(last and tap == 8))
                    nc.tensor.matmul(v1_ps[:], wv[:, tap * 128:tap * 128 + 128], win2,
                                     start=(first and tap == 0), stop=(last and tap == 8))

            k1_sb = opool.tile([128, 15 * 33], F16, name="k1_sb")
            k1o = k1_sb.rearrange("c (h w) -> c h w", h=15)
            k1g = k1_ps.rearrange("c (h w) -> c h w", h=16)
            _lrelu(nc, misc, k1g[:, 0:15, 0:31], bcol(1), bcol(11), k1o[:, :, 0:31], "k1e")
            nc.vector.tensor_copy(k1o[:, :, 31:33], k1o[:, :, 0:2])
            v1_sb = opool.tile([128, 15 * 33], F16, name="v1_sb")
            v1o = v1_sb.rearrange("c (h w) -> c h w", h=15)
            v1g = v1_ps.rearrange("c (h w) -> c h w", h=16)
            _lrelu(nc, misc, v1g[:, 0:15, 0:31], bcol(2), bcol(12), v1o[:, :, 0:31], "v1e")
            nc.vector.tensor_copy(v1o[:, :, 31:33], v1o[:, :, 0:2])

            # ---------------- q1 block machinery -----------------------------
            def q1_block(bi, r0, r1, tag, width):
                ps = ppool.tile([128, width], F32, name=f"q1ps{bi}", tag=tag,
                                bufs=(2 if tag == "pq1s" else 1))
                nrow = r1 - r0
                for ic in range(16):
                    xp = xpool.tile([128, (nrow + 2) * 66], F16,
                                    name=f"xp{bi}", tag="xq", bufs=2)
                    nc.sync.dma_start(xp[:], xpad_d.ap()[ic][:, r0 * 66:(r1 + 2) * 66])
                    x3 = xp.rearrange("c (h w) -> c h w", h=nrow + 2)
                    first, last = (ic == 0), (ic == 15)
                    for tap in range(9):
                        dy, dx = tap // 3, tap % 3
                        wq_t = w1q_sb[:, ic * 1152 + tap * 128: ic * 1152 + tap * 128 + 128]
                        for (c0, c1) in _rc(0, nrow):
                            win = x3[:, c0 + dy: c1 + dy, dx: dx + 64]
                            nc.tensor.matmul(ps[:, c0 * 64: c1 * 64], wq_t, win,
                                             start=(first and tap == 0),
                                             stop=(last and tap == 8))
                    yield ic
                # epilogue: q1o padded rows r0+1..r1, cols 1..65 (+wraps)
                pv = ps.rearrange("c (h w) -> c h w", h=width // 64)
                for (c0, c1) in _rc(0, nrow):
                    _lrelu(nc, misc, pv[:, c0:c1, :], bcol(0), bcol(10),
                           q1o[:, r0 + 1 + c0: r0 + 1 + c1, 1:65], f"q1e{bi}{c0}")
                nc.vector.tensor_copy(q1o[:, r0 + 1:r1 + 1, 0:1], q1o[:, r0 + 1:r1 + 1, 64:65])
                nc.vector.tensor_copy(q1o[:, r0 + 1:r1 + 1, 65:66], q1o[:, r0 + 1:r1 + 1, 1:2])
                if r0 == 0:
                    nc.vector.tensor_copy(q1o[:, 0:1, :], q1o[:, 2:3, :])
                if r1 == 32:
                    nc.vector.tensor_copy(q1o[:, 33:34, :], q1o[:, 31:32, :])

            def q2_block(bi):
                s0, s1 = Q2B[bi]
                npos = (s1 - s0) * 64
                for cc in range(2):
                    for (c0, c1) in _rc(s0, s1):
                        n = (c1 - c0) * 64
                        ps = ppool.tile([128, 512], F32, name=f"q2ps{bi}{cc}{c0}",
                                        tag="pq2", bufs=1)
                        for tap in range(9):
                            dy, dx = tap // 3, tap % 3
                            wslc = w2q_sb[:, tap * 256 + 128 * cc: tap * 256 + 128 * cc + 128]
                            win = q1o[:, c0 + dy: c1 + dy, dx: dx + 64]
                            nc.tensor.matmul(ps[:, 0:n], wslc, win,
                                             start=(tap == 0), stop=(tap == 8))
                        qps = misc.tile([128, 512], F16, name=f"qps{bi}{cc}{c0}",
                                        tag="rss", bufs=2)
                        nc.scalar.copy(qps[:, 0:n], ps[:, 0:n])
                        dst = rsb_in[bi][4 * cc:4 * cc + 4, :].rearrange(
                            "r (c p) -> r c p", c=32)[:, :, (c0 - s0) * 64:(c1 - s0) * 64]
                        nc.scalar.dma_start(dst, qps[:, 0:n])
                nc.gpsimd.collective_compute("ReduceScatter", ALU.add, replica_groups=RG,
                                             ins=[rsb_in[bi].opt()], outs=[rsb_out[bi].opt()])

            def q2_epi(bi):
                s0, s1 = Q2B[bi]
                npos = (s1 - s0) * 64
                q2r = misc.tile([32, npos], F16, name=f"q2r{bi}", tag="rsl", bufs=2)
                nc.gpsimd.dma_start(q2r[:], rsb_out[bi].rearrange("(c p) -> c p", c=32))
                q2rv = q2r.rearrange("c (h w) -> c h w", h=s1 - s0)
                _lrelu(nc, misc, q2rv, bcol(3)[0:32], bcol(13)[0:32],
                       q2o[:, s0 + 1:s1 + 1, 1:65], f"q2e{bi}")
                nc.vector.tensor_copy(q2o[:, s0 + 1:s1 + 1, 0:1], q2o[:, s0 + 1:s1 + 1, 64:65])
                nc.vector.tensor_copy(q2o[:, s0 + 1:s1 + 1, 65:66], q2o[:, s0 + 1:s1 + 1, 1:2])
                if s0 == 0:
                    nc.vector.tensor_copy(q2o[:, 0:1, :], q2o[:, 2:3, :])
                if s1 == 32:
                    nc.vector.tensor_copy(q2o[:, 33:34, :], q2o[:, 31:32, :])
                pr0, pr1 = PRB[bi]
                nc.scalar.dma_start(ag2b_in[bi][:], q2_sb[:, pr0 * 66:pr1 * 66])
                nc.gpsimd.collective_compute("AllGather", ALU.bypass, replica_groups=RG,
                                             ins=[ag2b_in[bi].opt()], outs=[ag2b_out[bi].opt()])

            def q2full_load(bi):
                pr0, pr1 = PRB[bi]
                nc.gpsimd.dma_start(q2f4[:, :, pr0:pr1, :],
                                    ag2b_out[bi].rearrange("(c p) f -> p c f", c=2))

            def q3_mms(bi):
                u0, u1 = Q3B[bi]
                npos = (u1 - u0) * 64
                q3l = misc.tile([32, npos], F16, name=f"q3l{bi}", tag="q3l", bufs=2)
                for (c0, c1) in _rc(u0, u1):
                    n = (c1 - c0) * 64
                    ps = ppool.tile([32, 512], F32, name=f"q3ps{bi}{c0}", tag="pq3", bufs=1)
                    for jc in range(2):
                        for tap in range(9):
                            dy, dx = tap // 3, tap % 3
                            w = w3q_sb[:, jc * 288 + tap * 32: jc * 288 + tap * 32 + 32]
                            win = q2f4[:, jc, c0 + dy: c1 + dy, dx: dx + 64]
                            nc.tensor.matmul(ps[:, 0:n], w, win,
                                             start=(jc == 0 and tap == 0),
                                             stop=(jc == 1 and tap == 8))
                    _lrelu(nc, misc, ps[:, 0:n], bcol(6)[0:32], bcol(16)[0:32],
                           q3l[:, (c0 - u0) * 64:(c1 - u0) * 64], f"q3e{bi}{c0}")
                nc.scalar.dma_start(ag3_in[bi][:], q3l[:])

            q3fs = {}

            def ag3_issue(bi):
                u0, u1 = Q3B[bi]
                npos = (u1 - u0) * 64
                nc.gpsimd.collective_compute("AllGather", ALU.bypass, replica_groups=RG,
                                             ins=[ag3_in[bi].opt()], outs=[ag3_out[bi].opt()])
                q3f = misc.tile([128, 2 * npos], F16, name=f"q3f{bi}", tag="q3f", bufs=2)
                nc.gpsimd.dma_start(q3f.rearrange("p (c f) -> p c f", c=2),
                                    ag3_out[bi].rearrange("(c p) f -> p c f", c=2))
                q3fs[bi] = q3f

            def scores_mms(bi):
                u0, u1 = Q3B[bi]
                npos = (u1 - u0) * 64
                col0 = u0 * 64
                q3fv = q3fs[bi].rearrange("p (c f) -> p c f", c=2)
                for o0 in range(0, npos, 512):
                    o1 = min(o0 + 512, npos)
                    ps = ppool.tile([65, 512], F32, name=f"sc{bi}{o0}", tag="pk", bufs=1)
                    for jc in range(2):
                        nc.tensor.matmul(ps[:, 0:o1 - o0], k3f[:, 65 * jc:65 * jc + 65],
                                         q3fv[:, jc, o0:o1],
                                         start=(jc == 0), stop=(jc == 1))
                    nc.scalar.copy(scores_sb[:, col0 + o0:col0 + o1], ps[:, 0:o1 - o0])
                nc.vector.reduce_max(pmax[bi][:], scores_sb[:, col0:col0 + npos], axis=AX.X)

            # ================ schedule ======================================
            # q1 block A with k-chain interleaved at ic boundaries
            gA = q1_block(0, 0, 16, "pq1a", 1024)
            next(gA)  # ic0

            # --- k2/v2 partials (depend on k1o/v1o epilogues) ---
            for cc in range(2):
                kp = ppool.tile([128, 112], F32, name="kp", tag="pk")
                for tap in range(9):
                    dy, dx = tap // 3, tap % 3
                    wink = k1o[:, dy: dy + 13: 2, dx: dx + 31: 2]
                    nc.tensor.matmul(kp[:], w2k_sb[:, tap * 256 + 128 * cc: tap * 256 + 128 * cc + 128],
                                     wink, start=(tap == 0), stop=(tap == 8))
                kps = misc.tile([128, 112], F16, name="kps", tag="rss2", bufs=2)
                nc.scalar.copy(kps[:], kp[:])
                dst = rsa_in[4 * cc:4 * cc + 4, 0:3584].rearrange("r (c p) -> r c p", c=32)
                nc.scalar.dma_start(dst, kps[:])
            for cc in range(8):
                vp = ppool.tile([128, 112], F32, name="vp", tag="pv")
                for tap in range(9):
                    dy, dx = tap // 3, tap % 3
                    winv = v1o[:, dy: dy + 13: 2, dx: dx + 31: 2]
                    nc.tensor.matmul(vp[:], w2v_sb[:, 4608 * (cc // 4) + tap * 512 + 128 * (cc % 4):
                                               4608 * (cc // 4) + tap * 512 + 128 * (cc % 4) + 128],
                                     winv, start=(tap == 0), stop=(tap == 8))
                vps = misc.tile([128, 112], F16, name="vps", tag="rss2", bufs=2)
                nc.scalar.copy(vps[:], vp[:])
                nc.scalar.dma_start(rsa_in[cc, 3584:17920].rearrange("(c p) -> c p", c=128), vps[:])
            nc.gpsimd.collective_compute("ReduceScatter", ALU.add, replica_groups=RG,
                                         ins=[rsa_in.opt()], outs=[rsa_out.opt()])

            # k2/v2 shard epilogues + ag2a
            k2r = misc.tile([32, 112], F16, name="k2r", tag="rsl0")
            nc.gpsimd.dma_start(k2r[:], rsa_out[0:3584].rearrange("(c p) -> c p", c=32))
            v2r = misc.tile([128, 112], F16, name="v2r", tag="rsl2")
            nc.gpsimd.dma_start(v2r[:], rsa_out[3584:17920].rearrange("(c p) -> c p", c=128))
            k2_sb = opool.tile([32, 112], F16, name="k2_sb")
            k2o = k2_sb.rearrange("c (h w) -> c h w", h=7)
            k2rg = k2r.rearrange("c (h w) -> c h w", h=7)
            _lrelu(nc, misc, k2rg[:, :, 0:15], bcol(4)[0:32], bcol(14)[0:32], k2o[:, :, 0:15], "k2e")
            nc.vector.tensor_copy(k2o[:, :, 15:16], k2o[:, :, 0:1])
            v2_sb = opool.tile([128, 112], F16, name="v2_sb")
            v2o = v2_sb.rearrange("c (h w) -> c h w", h=7)
            v2rg = v2r.rearrange("c (h w) -> c h w", h=7)
            _lrelu(nc, misc, v2rg[:, :, 0:15], bcol(5), bcol(15), v2o[:, :, 0:15], "v2e")
            nc.vector.tensor_copy(v2o[:, :, 15:16], v2o[:, :, 0:1])
            nc.scalar.dma_start(ag2a_in[0:3584].rearrange("(c p) -> c p", c=32), k2_sb[:])
            nc.scalar.dma_start(ag2a_in[3584:17920].rearrange("(c p) -> c p", c=128), v2_sb[:])
            nc.gpsimd.collective_compute("AllGather", ALU.bypass, replica_groups=RG,
                                         ins=[ag2a_in.opt()], outs=[ag2a_out.opt()])

            for ic in range(1, 7):
                next(gA)

            # --- k3/v3 (need ag2a) ---
            k3_ps = ppool.tile([32, 70], F32, name="k3_ps", tag="pk")
            v3_ps = ppool.tile([128, 70], F32, name="v3_ps", tag="pv")
            k2rr = ag2a_out[:, 0:3584].rearrange("r (c p) -> r c p", c=32)
            v2rr = ag2a_out[:, 3584:17920].rearrange("r (c p) -> r c p", c=128)
            for jc in range(2):
                k2c = xpool.tile([128, 112], F16, name="k2c", tag="k2c")
                nc.gpsimd.dma_start(k2c[:], k2rr[4 * jc:4 * jc + 4])
                k2c3 = k2c.rearrange("c (h w) -> c h w", h=7)
                for tap in range(9):
                    dy, dx = tap // 3, tap % 3
                    wink = k2c3[:, dy: dy + 5, dx: dx + 14]
                    nc.tensor.matmul(k3_ps[:], w3k_sb[:, jc * 288 + tap * 32:jc * 288 + tap * 32 + 32],
                                     wink, start=(jc == 0 and tap == 0), stop=(jc == 1 and tap == 8))
            for ic in range(8):
                v2c = xpool.tile([128, 112], F16, name="v2c", tag="v2c")
                nc.gpsimd.dma_start(v2c[:], v2rr[ic])
                v2c3 = v2c.rearrange("c (h w) -> c h w", h=7)
                for tap in range(9):
                    dy, dx = tap // 3, tap % 3
                    winv = v2c3[:, dy: dy + 5, dx: dx + 14]
                    nc.tensor.matmul(v3_ps[:], w3v_sb[:, ic * 1152 + tap * 128:ic * 1152 + tap * 128 + 128],
                                     winv, start=(ic == 0 and tap == 0), stop=(ic == 7 and tap == 8))

            k3g = k3_ps.rearrange("c (h w) -> c h w", h=5)
            k3_sb = opool.tile([32, 65], F16, name="k3_sb")
            _lrelu(nc, misc, k3g[:, :, 0:13], bcol(7)[0:32], bcol(17)[0:32], k3_sb[:], "k3e")
            v3g = v3_ps.rearrange("c (h w) -> c h w", h=5)
            v3_sb = opool.tile([128, 65], F16, name="v3_sb")
            _lrelu(nc, misc, v3g[:, :, 0:13], bcol(8), bcol(18), v3_sb[:], "v3e")
            nc.scalar.dma_start(agkv_in[:, 0:65], k3_sb[:])
            nc.scalar.dma_start(agkv_in[:, 65:325].rearrange("c (a p) -> c a p", a=4), v3_sb[:])
            nc.gpsimd.collective_compute("AllGather", ALU.bypass, replica_groups=RG,
                                         ins=[agkv_in.opt()], outs=[agkv_out.opt()])

            for ic in range(7, 11):
                next(gA)

            # --- pv3 = pw @ v3 (replicated; needs agkv) + transpose ----------
            nc.gpsimd.dma_start(k3f.rearrange("p (c f) -> p c f", c=2),
                                agkv_out[:, 0:65].rearrange("(c p) f -> p c f", c=2))
            for i in range(8):
                nc.gpsimd.dma_start(v3fa[:, 65 * i:65 * i + 65],
                                    agkv_out[32 * i:32 * i + 32, 65:325].rearrange(
                                        "r (a p) -> r a p", a=4))
            for j in range(8):
                ppv = ppool.tile([128, 65], F32, name=f"ppv{j}", tag="pv", bufs=1)
                for i in range(8):
                    nc.tensor.matmul(ppv[:], wp_sb[:, 1024 * i + 128 * j: 1024 * i + 128 * j + 128],
                                     v3fa[:, 65 * i:65 * i + 65],
                                     start=(i == 0), stop=(i == 7))
                nc.scalar.copy(pv3sb[:, 65 * j:65 * j + 65], ppv[:])
            for j in range(8):
                tps = ppool.tile([65, 128], F32, name=f"tps{j}", tag="pk", bufs=1)
                nc.tensor.transpose(tps[:], pv3sb[:, 65 * j:65 * j + 65], ident[:])
                nc.scalar.copy(pv3T[:, 128 * j:128 * j + 128], tps[:])

            for ic in range(11, 16):
                next(gA)
            for _ in gA:  # epilogue A
                pass

            # ---------------- q1 block B + chain A ---------------------------
            gB = q1_block(1, 16, 24, "pq1s", 512)
            next(gB)
            next(gB)
            q2_block(0)
            for ic in range(2, 16):
                next(gB)
            q2_epi(0)
            q2full_load(0)
            for _ in gB:
                pass

            # ---------------- q1 block C + chain A/B -------------------------
            gC = q1_block(2, 24, 32, "pq1s", 512)
            next(gC)
            next(gC)
            q2_block(1)
            for ic in range(2, 5):
                next(gC)
            q3_mms(0)
            for ic in range(5, 11):
                next(gC)
            q2_epi(1)
            q2full_load(1)
            ag3_issue(0)
            for ic in range(11, 16):
                next(gC)
            scores_mms(0)
            for _ in gC:
                pass

            # ---------------- tail: chain B/C + softmax + out -----------------
            q3_mms(1)
            ag3_issue(1)
            q2_block(2)
            scores_mms(1)
            q2_epi(2)
            q2full_load(2)
            q3_mms(2)
            ag3_issue(2)
            scores_mms(2)

            # softmax over query axis (replicated)
            gmax = misc.tile([65, 1], F32, name="gmax")
            nc.vector.tensor_tensor(gmax[:], pmax[0][:], pmax[1][:], op=ALU.max)
            nc.vector.tensor_tensor(gmax[:], gmax[:], pmax[2][:], op=ALU.max)
            negmax = misc.tile([65, 1], F32, name="negmax")
            nc.scalar.activation(negmax[:], gmax[:], AF.Identity, scale=-1.0)
            esum = misc.tile([65, 1], F32, name="esum")
            bexp = misc.tile([65, 2048], F16, name="bexp")
            nc.scalar.activation(bexp[:], scores_sb[:], AF.Exp, bias=negmax[:, 0:1],
                                 accum_out=esum[:, 0:1])
            rsum = misc.tile([65, 1], F32, name="rsum")
            nc.vector.reciprocal(rsum[:], esum[:])
            nc.gpsimd.dma_start(beta_dram[:], bexp[:])

            # indirect gather of MY 256 beta columns (row (m, blk) of (520, 256))
            betaB = misc.tile([65, 256], F16, name="betaB")
            nc.gpsimd.indirect_dma_start(
                out=betaB[:], out_offset=None,
                in_=beta_dram.rearrange("m (b p) -> (m b) p", b=8),
                in_offset=bass.IndirectOffsetOnAxis(ap=bidx[:, 0:1], axis=0))
            nc.vector.tensor_scalar_mul(betaB[:], betaB[:], rsum[:, 0:1])

            # out = pv3T @ betaB + pb  (position shard: 256 cols)
            for cc in range(8):
                ops = ppool.tile([128, 256], F32, name=f"ops{cc}", tag="pv", bufs=1)
                nc.tensor.matmul(ops[:], pv3T[:, 128 * cc:128 * cc + 128], betaB[:],
                                 start=True, stop=True)
                out_sb = misc.tile([128, 256], F32, name=f"out_sb{cc}", tag="osb", bufs=2)
                nc.vector.tensor_scalar_add(out_sb[:], ops[:], bcol(20 + cc))
                nc.sync.dma_start(out_d.ap()[128 * cc:128 * cc + 128], out_sb[:])

    nc.compile()
    nc.m = get_hw_module(nc.m)
    return nc


def _prep_inputs(x, qw1, qb1, qw2, qb2, qw3, qb3, kw1, kb1, kw2, kb2, kw3, kb3,
                 vw1, vb1, vw2, vb2, vw3, vb3, pw, pb):
    f = np.float32
    h = np.float16
    x = np.ascontiguousarray(np.asarray(x).reshape(2048, 32, 64), dtype=f)
    xp = np.concatenate([x[:, 1:2], x, x[:, 30:31]], axis=1)
    xp = np.concatenate([xp[:, :, -1:], xp, xp[:, :, :1]], axis=2)
    xpad = np.ascontiguousarray(xp.reshape(16, 128, 34 * 66), dtype=h)
    xdec = np.zeros((16, 128, 4, 17, 33), h)
    xr = x.reshape(16, 128, 32, 64)
    for py in range(2):
        for px in range(2):
            xdec[:, :, 2 * py + px, 0:16, 0:32] = xr[:, :, py::2, px::2]
    xdec = np.ascontiguousarray(xdec.reshape(16, 128, 4 * 17 * 33))

    def conv_w(wt, co_lo, co_n, nchunk):
        ws = np.asarray(wt)[co_lo:co_lo + co_n]           # (co_n, Ci, 3, 3)
        ci = ws.shape[1]
        a = ws.reshape(co_n, nchunk, ci // nchunk, 9)     # (co, ck, ci, tap)
        a = a.transpose(1, 2, 3, 0)                       # (ck, ci, tap, co)
        return np.ascontiguousarray(a.reshape(nchunk, ci // nchunk, 9 * co_n), dtype=h)

    def conv_w_ci(wt, ci_lo):
        ws = np.asarray(wt)[:, ci_lo:ci_lo + 128]         # (co, 128, 3, 3)
        co = ws.shape[0]
        a = ws.reshape(co, 128, 9).transpose(1, 2, 0)     # (ci, tap, co)
        return np.ascontiguousarray(a.reshape(128, 9 * co), dtype=h)

    in_maps = []
    for c in range(NCORES):
        m = {"xpad": xpad, "xdec": xdec}
        m["w1q"] = conv_w(qw1, 128 * c, 128, 16)
        m["w1k"] = conv_w(kw1, 128 * c, 128, 16)
        m["w1v"] = conv_w(vw1, 128 * c, 128, 16)
        m["w2q"] = conv_w_ci(qw2, 128 * c)
        m["w2k"] = conv_w_ci(kw2, 128 * c)
        wv2 = np.asarray(vw2)[:, 128 * c:128 * c + 128]        # (1024co, 128ci, 3, 3)
        wv2 = wv2.reshape(2, 512, 128, 9).transpose(0, 2, 3, 1)  # (half, ci, tap, co512)
        m["w2v"] = np.ascontiguousarray(wv2.reshape(2, 128, 4608), dtype=h)
        m["w3q"] = conv_w(qw3, 32 * c, 32, 2)
        m["w3k"] = conv_w(kw3, 32 * c, 32, 2)
        m["w3v"] = conv_w(vw3, 128 * c, 128, 8)
        m["wp"] = np.ascontiguousarray(
            np.asarray(pw)[:, :, 0, 0].T.reshape(8, 128, 1024), dtype=h)
        bias = np.zeros((128, 28), f)
        bias[:, 0] = qb1[128 * c:128 * c + 128]
        bias[:, 1] = kb1[128 * c:128 * c + 128]
        bias[:, 2] = vb1[128 * c:128 * c + 128]
        bias[0:32, 3] = qb2[32 * c:32 * c + 32]
        bias[0:32, 4] = kb2[32 * c:32 * c + 32]
        bias[:, 5] = vb2[128 * c:128 * c + 128]
        bias[0:32, 6] = qb3[32 * c:32 * c + 32]
        bias[0:32, 7] = kb3[32 * c:32 * c + 32]
        bias[:, 8] = vb3[128 * c:128 * c + 128]
        bias[:, 10:19] = 0.3 * bias[:, 0:9]
        for j in range(8):
            bias[:, 20 + j] = pb[128 * j:128 * j + 128]
        m["bias"] = bias
        m["bidx"] = np.arange(65, dtype=np.int32).reshape(65, 1) * 8 + c
        in_maps.append(m)
    return in_maps


LAST_RESULT = None


def kernel(**inputs):
    global LAST_RESULT
    if "nc" not in _CACHE:
        _CACHE["nc"] = build_program()
    nc = _CACHE["nc"]
    in_maps = _prep_inputs(**{k: np.asarray(v) for k, v in inputs.items()})
    res = bass_utils.run_bass_kernel_spmd(nc, in_maps, core_ids=list(range(NCORES)))
    LAST_RESULT = res
    out = np.empty((1024, 32, 64), np.float32)
    for c in range(NCORES):
        out[:, 4 * c:4 * c + 4, :] = res.results[c]["out_shard"].reshape(1024, 4, 64)
    return np.ascontiguousarray(out.reshape(1, 1024, 32, 64))
